# revision 8
# baseline (speedup 1.0000x reference)
"""Trainium2 Bass kernel for nn_ActionDetokenizer (gnn_message_passing).

Computes: out[b, j, k] = sum_d x[b, j+1, d] * W[j, d, k] + bias[j, k]
  x: [65536, 13, 256] f32, W: [12, 256, 2] f32, b: [12, 2] f32 -> out [65536, 12, 2] f32

Strategy (pure data parallel over batch, 8 cores):
  - Host: shard batch across 8 cores; per core, relayout the needed slice of x
    to d-major [12, C*128, 8192] so the contraction dim (d) lands on SBUF
    partitions (the TensorEngine contracts along partitions).  The tiny weight
    stack is replicated to every core.
  - Device: stream x tiles HBM->SBUF (memory-bound: ~100.7 MB/core) as one
    2 MiB DMA per (joint, batch-chunk); for each (joint, 512-batch column
    chunk) accumulate K=128-contraction matmuls into PSUM with W[j] chunks as
    the stationary operand.  ScalarEngine drains PSUM->SBUF fusing the
    per-(j,k) bias add (Identity activation with per-partition bias AP).
    Output is written k-major [12, 2, 8192] per core and re-oriented on the
    host during the gather step.  Output DMAs ride the second HWDGE ring
    (ACT) so they never stall the input ring (SP).

Compute paths (KERNEL_PATH env or _PATH):
  f32  : exact fp32 matmuls (4 cyc/row on PE)
  f32r : float32r matmuls (1 cyc/row at N>=256; ~1.4e-4 scale-rel err on HW)
  hilo : x and W split into bf16 hi+lo on host; 3-term bf16 matmuls
         (xhi@whi + xhi@wlo + xlo@whi), ~5e-6 scale-rel err, 1 cyc/row.
"""

import os

import numpy as np

M_CORES = 8
B_FULL = 65536
BL = B_FULL // M_CORES  # 8192 batch rows per core
J = 12  # joints
D = 256  # embed dim
K = 2  # outputs per joint
P = 128  # SBUF partitions / d-chunk
C = D // P  # 2 d-chunks
NB_TILE = 4096  # batch columns per SBUF x tile
OT_TILE = 2048  # batch columns per output staging tile
N_MM = 512  # batch columns per matmul (fp32 moving-operand max / PSUM bank)

_PATH = os.environ.get("KERNEL_PATH", "f32")

_CACHE = {}


def _build(path, bl):
    import concourse.bacc as bacc
    import concourse.mybir as mybir
    from concourse.tile import TileContext

    f32 = mybir.dt.float32
    xdt = {
        "f32": f32,
        "f32r": mybir.dt.float32r,
        "hilo": mybir.dt.bfloat16,
    }[path]
    # number of (term) planes stacked along the x free dim per joint:
    # f32/f32r: C d-chunks; hilo: 2 sources (hi, lo) x C d-chunks
    n_src = 2 if path == "hilo" else 1
    planes = n_src * C

    # Bacc (not plain Bass): its compile() legalizes multi-wait instructions
    # into event semaphores / ldweights waits, which walrus codegen requires
    # (at most one wait command per compute instruction on TRN2).
    nc = bacc.Bacc("TRN2", target_bir_lowering=False, dynamic_dma_scratch_size=2048)

    # x relayout per core: [J, planes*P, bl]
    x_dram = nc.dram_tensor("xt", [J, planes * P, bl], xdt, kind="ExternalInput")
    # weights: [P, J*n_wsrc*C*K]; hilo has whi,wlo stacked along free dim
    n_wsrc = 2 if path == "hilo" else 1
    w_dram = nc.dram_tensor(
        "wt", [P, J * n_wsrc * C * K], xdt, kind="ExternalInput"
    )
    bias_dram = nc.dram_tensor("bias", [K, J], f32, kind="ExternalInput")
    out_dram = nc.dram_tensor("out", [J, K, bl], f32, kind="ExternalOutput")

    nb = min(NB_TILE, bl)
    assert bl % nb == 0 and nb % N_MM == 0
    n_bh = bl // nb
    n_n = nb // N_MM

    # matmul term sequence per (j, n): (w_src, x_src, c)
    if path == "hilo":
        # hi@whi, lo@whi, hi@wlo  (xlo@wlo dropped: ~2^-18 relative)
        terms = [(0, 0), (0, 1), (1, 0)]
    else:
        terms = [(0, 0)]
    mms = [(ws, xs, c) for (ws, xs) in terms for c in range(C)]

    def w_col(j, ws, c):
        # column offset of W chunk (j, ws, c) in w_dram/w_sb
        return ((j * n_wsrc + ws) * C + c) * K

    def x_plane(xs, c):
        return xs * C + c

    with TileContext(nc) as tc:
        with (
            tc.tile_pool(name="wpool", bufs=1) as wpool,
            tc.tile_pool(name="xpool", bufs=3) as xpool,
            tc.tile_pool(name="opool", bufs=3) as opool,
            tc.tile_pool(name="pspool", bufs=8, space="PSUM") as pspool,
        ):
            w_sb = wpool.tile([P, J * n_wsrc * C * K], xdt, tag="w")
            nc.sync.dma_start(out=w_sb[:, :], in_=w_dram[:, :])
            bias_sb = wpool.tile([K, J], f32, tag="bias")
            nc.sync.dma_start(out=bias_sb[:, :], in_=bias_dram[:, :])

            for j in range(J):
                for bh in range(n_bh):
                    # one DMA brings all planes (d-chunks x hi/lo) for this
                    # (j, bh): [P, planes*nb], 2 MiB
                    xt = xpool.tile([P, planes * nb], xdt, tag="x")
                    src = x_dram[j, :, bh * nb : (bh + 1) * nb]
                    nc.sync.dma_start(
                        out=xt.rearrange("p (pl b) -> p pl b", pl=planes),
                        in_=src.rearrange("(pl p) b -> p pl b", p=P),
                    )
                    ot_chunks = nb // OT_TILE
                    n_per_ot = OT_TILE // N_MM
                    for oh in range(ot_chunks):
                        ot = opool.tile([K, OT_TILE], f32, tag="o")
                        for nn in range(n_per_ot):
                            n = oh * n_per_ot + nn
                            ps = pspool.tile([K, N_MM], f32, tag="ps")
                            for i, (ws, xs, c) in enumerate(mms):
                                pl = x_plane(xs, c)
                                col = pl * nb + n * N_MM
                                wc = w_col(j, ws, c)
                                nc.tensor.matmul(
                                    ps[:, :],
                                    lhsT=w_sb[:, wc : wc + K],
                                    rhs=xt[:, col : col + N_MM],
                                    start=(i == 0),
                                    stop=(i == len(mms) - 1),
                                )
                            # PSUM -> SBUF with fused per-(j,k) bias add,
                            # alternating ACT / DVE to split the drain work
                            osl = ot[:, nn * N_MM : (nn + 1) * N_MM]
                            if n % 2 == 0:
                                nc.scalar.activation(
                                    out=osl,
                                    in_=ps[:, :],
                                    func=mybir.ActivationFunctionType.Identity,
                                    bias=bias_sb[:, j : j + 1],
                                    scale=1.0,
                                )
                            else:
                                nc.vector.tensor_scalar_add(
                                    out=osl,
                                    in0=ps[:, :],
                                    scalar1=bias_sb[:, j : j + 1],
                                )
                        # output DMA on the ACT HWDGE ring (keeps SP for x)
                        nc.scalar.dma_start(
                            out=out_dram[
                                j,
                                :,
                                bh * nb + oh * OT_TILE : bh * nb
                                + (oh + 1) * OT_TILE,
                            ],
                            in_=ot[:, :],
                        )
    nc.compile()
    return nc


def _get_nc(path, bl):
    key = (path, bl)
    if key not in _CACHE:
        _CACHE[key] = _build(path, bl)
    return _CACHE[key]


def _split_hilo(a):
    import ml_dtypes

    hi = a.astype(ml_dtypes.bfloat16)
    lo = (a - hi.astype(np.float32)).astype(ml_dtypes.bfloat16)
    return hi, lo


def _prep_core_inputs(x, W, b, path, n_cores, bl):
    """Shard batch across cores; relayout x slice to [J, planes*P, bl]."""
    # W chunks: [P, J*n_wsrc*C*K], wt[d, ((j*n_wsrc+ws)*C+c)*K + k]
    wt32 = W.reshape(J, C, P, K).transpose(2, 0, 1, 3)  # [P, J, C, K]
    if path == "hilo":
        hi, lo = _split_hilo(np.ascontiguousarray(wt32))  # [P, J, C, K] each
        wt = np.stack([hi, lo], axis=2)  # [P, J, 2, C, K]
        wt = np.ascontiguousarray(wt.reshape(P, J * 2 * C * K))
    else:
        wt = np.ascontiguousarray(wt32.reshape(P, J * C * K))
    bias = np.ascontiguousarray(b.T)  # [K, J]

    in_maps = []
    for m in range(n_cores):
        xs = x[m * bl : (m + 1) * bl, 1 : J + 1, :]  # [bl, J, D] view
        # -> [J, D, bl] = [J, C*P, bl]
        xt = np.ascontiguousarray(xs.transpose(1, 2, 0))
        if path == "hilo":
            hi, lo = _split_hilo(xt)  # [J, C*P, bl] each
            # planes per j: [hi_c0, hi_c1, lo_c0, lo_c1] along the P-axis
            xt = np.concatenate(
                [hi.reshape(J, C * P, bl), lo.reshape(J, C * P, bl)], axis=1
            )
        in_maps.append({"xt": xt, "wt": wt, "bias": bias})
    return in_maps


def _gather(results, n_cores, bl):
    # per-core out [J, K, bl] -> full [B, J, K]
    out = np.empty((n_cores * bl, J, K), dtype=np.float32)
    for m, r in enumerate(results):
        out[m * bl : (m + 1) * bl] = r["out"].transpose(2, 0, 1)
    return out


def _ensure_ntff_hook():
    """The agent image's antenv lacks axon_hooks; shim it so trace=True can
    register the NTFF profiling hook (see trn_agent_boot.trn_boot)."""
    import sys
    import types

    try:
        from antenv.axon_hooks import get_axon_ntff_profile_hook  # noqa: F401

        return
    except ImportError:
        pass
    import antenv

    mod = types.ModuleType("antenv.axon_hooks")
    mod._hook = None

    def set_axon_ntff_profile_hook(h):
        mod._hook = h

    def get_axon_ntff_profile_hook():
        return mod._hook

    mod.set_axon_ntff_profile_hook = set_axon_ntff_profile_hook
    mod.get_axon_ntff_profile_hook = get_axon_ntff_profile_hook
    sys.modules["antenv.axon_hooks"] = mod
    antenv.axon_hooks = mod
    try:
        from trn_agent_boot.trn_boot import _ntff_profile_via_ctypes

        hook = _ntff_profile_via_ctypes("/opt/axon/libaxon_pjrt.so")
        if hook is not None:
            mod._hook = hook
    except Exception:
        pass


def run(x, W, b, path=None, trace=False, n_cores=M_CORES, bl=None):
    from concourse.bass_utils import run_bass_kernel_spmd

    if trace:
        _ensure_ntff_hook()

    path = path or _PATH
    bl = bl or (x.shape[0] // n_cores)
    x = np.asarray(x, dtype=np.float32)
    W = np.asarray(W, dtype=np.float32)
    b = np.asarray(b, dtype=np.float32)
    nc = _get_nc(path, bl)
    in_maps = _prep_core_inputs(x, W, b, path, n_cores, bl)
    res = run_bass_kernel_spmd(
        nc, in_maps, core_ids=list(range(n_cores)), trace=trace
    )
    out = _gather(res.results, n_cores, bl)
    return out, res


def kernel(x, W, b):
    out, _ = run(x, W, b)
    return out


# revision 10
# speedup vs baseline: 1.2439x; 1.2439x over previous
"""Trainium2 Bass kernel for nn_ActionDetokenizer (gnn_message_passing).

Computes: out[b, j, k] = sum_d x[b, j+1, d] * W[j, d, k] + bias[j, k]
  x: [65536, 13, 256] f32, W: [12, 256, 2] f32, b: [12, 2] f32 -> out [65536, 12, 2] f32

Strategy (pure data parallel over batch, 8 cores):
  - Host: shard batch across 8 cores; per core, relayout the needed slice of x
    to d-major [12, C*128, 8192] so the contraction dim (d) lands on SBUF
    partitions (the TensorEngine contracts along partitions).  The tiny weight
    stack is replicated to every core.
  - Device: stream x tiles HBM->SBUF (memory-bound: ~100.7 MB/core) as one
    2 MiB DMA per (joint, batch-chunk); for each (joint, 512-batch column
    chunk) accumulate K=128-contraction matmuls into PSUM with W[j] chunks as
    the stationary operand.  ScalarEngine drains PSUM->SBUF fusing the
    per-(j,k) bias add (Identity activation with per-partition bias AP).
    Output is written k-major [12, 2, 8192] per core and re-oriented on the
    host during the gather step.  Output DMAs ride the second HWDGE ring
    (ACT) so they never stall the input ring (SP).

Compute paths (KERNEL_PATH env or _PATH):
  f32  : exact fp32 matmuls (4 cyc/row on PE)
  f32r : float32r matmuls (1 cyc/row at N>=256; ~1.4e-4 scale-rel err on HW)
  hilo : x and W split into bf16 hi+lo on host; 3-term bf16 matmuls
         (xhi@whi + xhi@wlo + xlo@whi), ~5e-6 scale-rel err, 1 cyc/row.
"""

import os

import numpy as np

M_CORES = 8
B_FULL = 65536
BL = B_FULL // M_CORES  # 8192 batch rows per core
J = 12  # joints
D = 256  # embed dim
K = 2  # outputs per joint
P = 128  # SBUF partitions / d-chunk
C = D // P  # 2 d-chunks
NB_TILE = 4096  # batch columns per SBUF x tile
OT_TILE = 2048  # batch columns per output staging tile
N_MM = 512  # batch columns per matmul (fp32 moving-operand max / PSUM bank)
G = 4  # column-tiling stripes (concurrent matmuls at PE col groups 32*g)

_PATH = os.environ.get("KERNEL_PATH", "f32")

_CACHE = {}


def _build(path, bl):
    import concourse.bacc as bacc
    import concourse.mybir as mybir
    from concourse.tile import TileContext

    f32 = mybir.dt.float32
    xdt = {
        "f32": f32,
        "f32r": mybir.dt.float32r,
        "hilo": mybir.dt.bfloat16,
    }[path]
    # number of (term) planes stacked along the x free dim per joint:
    # f32/f32r: C d-chunks; hilo: 2 sources (hi, lo) x C d-chunks
    n_src = 2 if path == "hilo" else 1
    planes = n_src * C

    # Bacc (not plain Bass): its compile() legalizes multi-wait instructions
    # into event semaphores / ldweights waits, which walrus codegen requires
    # (at most one wait command per compute instruction on TRN2).
    nc = bacc.Bacc("TRN2", target_bir_lowering=False, dynamic_dma_scratch_size=2048)

    # x relayout per core: [J, planes*P, bl]
    x_dram = nc.dram_tensor("xt", [J, planes * P, bl], xdt, kind="ExternalInput")
    # weights: [P, J*n_wsrc*C*K]; hilo has whi,wlo stacked along free dim
    n_wsrc = 2 if path == "hilo" else 1
    w_dram = nc.dram_tensor(
        "wt", [P, J * n_wsrc * C * K], xdt, kind="ExternalInput"
    )
    bias_dram = nc.dram_tensor("bias", [K, J], f32, kind="ExternalInput")

    nb = min(NB_TILE, bl)
    assert bl % nb == 0 and nb % N_MM == 0
    n_bh = bl // nb
    n_n = nb // N_MM
    g_n = min(G, n_n)  # stripes actually used

    out_dram = nc.dram_tensor(
        "out", [J, g_n, K, bl // g_n], f32, kind="ExternalOutput"
    )

    # matmul term sequence per (j, n): (w_src, x_src, c)
    if path == "hilo":
        # hi@whi, lo@whi, hi@wlo  (xlo@wlo dropped: ~2^-18 relative)
        terms = [(0, 0), (0, 1), (1, 0)]
    else:
        terms = [(0, 0)]
    mms = [(ws, xs, c) for (ws, xs) in terms for c in range(C)]

    def w_col(j, ws, c):
        # column offset of W chunk (j, ws, c) in w_dram/w_sb
        return ((j * n_wsrc + ws) * C + c) * K

    def x_plane(xs, c):
        return xs * C + c

    with TileContext(nc) as tc:
        with (
            tc.tile_pool(name="wpool", bufs=1) as wpool,
            tc.tile_pool(name="xpool", bufs=3) as xpool,
            tc.tile_pool(name="opool", bufs=3) as opool,
            tc.tile_pool(name="pspool", bufs=8, space="PSUM") as pspool,
        ):
            w_sb = wpool.tile([P, J * n_wsrc * C * K], xdt, tag="w")
            nc.sync.dma_start(out=w_sb[:, :], in_=w_dram[:, :])
            bias_sb = wpool.tile([K, J], f32, tag="bias")
            nc.sync.dma_start(out=bias_sb[:, :], in_=bias_dram[:, :])

            n_grp = n_n // g_n  # n-chunk groups per (j, bh)
            for j in range(J):
                for bh in range(n_bh):
                    # one DMA brings all planes (d-chunks x hi/lo) for this
                    # (j, bh): [P, planes*nb]
                    xt = xpool.tile([P, planes * nb], xdt, tag="x")
                    src = x_dram[j, :, bh * nb : (bh + 1) * nb]
                    nc.sync.dma_start(
                        out=xt.rearrange("p (pl b) -> p pl b", pl=planes),
                        in_=src.rearrange("(pl p) b -> p pl b", p=P),
                    )
                    # stripe layout: PSUM/SBUF partition rows 32*g hold the
                    # output of n-chunk n = grp*G + g; the G stripes' matmuls
                    # run CONCURRENTLY in disjoint PE column groups.
                    ot = opool.tile([P, n_grp * N_MM], f32, tag="o")
                    for grp in range(n_grp):
                        ps = pspool.tile([P, N_MM], f32, tag="ps")
                        for i, (ws, xs, c) in enumerate(mms):
                            pl = x_plane(xs, c)
                            wc = w_col(j, ws, c)
                            for g in range(g_n):
                                n = grp * g_n + g
                                col = pl * nb + n * N_MM
                                nc.tensor.matmul(
                                    ps[32 * g : 32 * g + K, :],
                                    lhsT=w_sb[:, wc : wc + K],
                                    rhs=xt[:, col : col + N_MM],
                                    start=(i == 0),
                                    stop=(i == len(mms) - 1),
                                    tile_position=(0, 32 * g),
                                )
                        # PSUM -> SBUF with fused per-(j,k) bias add,
                        # alternating ACT / DVE to split the drain work
                        for g in range(g_n):
                            psl = ps[32 * g : 32 * g + K, :]
                            osl = ot[
                                32 * g : 32 * g + K,
                                grp * N_MM : (grp + 1) * N_MM,
                            ]
                            if g % 2 == 0:
                                nc.scalar.activation(
                                    out=osl,
                                    in_=psl,
                                    func=mybir.ActivationFunctionType.Identity,
                                    bias=bias_sb[:, j : j + 1],
                                    scale=1.0,
                                )
                            else:
                                nc.vector.tensor_scalar_add(
                                    out=osl,
                                    in0=psl,
                                    scalar1=bias_sb[:, j : j + 1],
                                )
                    # output DMAs on the ACT HWDGE ring (keeps SP for x):
                    # one per stripe; host unscrambles the stripe layout.
                    for g in range(g_n):
                        nc.scalar.dma_start(
                            out=out_dram[
                                j,
                                g,
                                :,
                                bh * n_grp * N_MM : (bh + 1) * n_grp * N_MM,
                            ],
                            in_=ot[32 * g : 32 * g + K, :],
                        )
    nc.compile()
    return nc


def _get_nc(path, bl):
    key = (path, bl)
    if key not in _CACHE:
        _CACHE[key] = _build(path, bl)
    return _CACHE[key]


def _split_hilo(a):
    import ml_dtypes

    hi = a.astype(ml_dtypes.bfloat16)
    lo = (a - hi.astype(np.float32)).astype(ml_dtypes.bfloat16)
    return hi, lo


def _prep_core_inputs(x, W, b, path, n_cores, bl):
    """Shard batch across cores; relayout x slice to [J, planes*P, bl]."""
    # W chunks: [P, J*n_wsrc*C*K], wt[d, ((j*n_wsrc+ws)*C+c)*K + k]
    wt32 = W.reshape(J, C, P, K).transpose(2, 0, 1, 3)  # [P, J, C, K]
    if path == "hilo":
        hi, lo = _split_hilo(np.ascontiguousarray(wt32))  # [P, J, C, K] each
        wt = np.stack([hi, lo], axis=2)  # [P, J, 2, C, K]
        wt = np.ascontiguousarray(wt.reshape(P, J * 2 * C * K))
    else:
        wt = np.ascontiguousarray(wt32.reshape(P, J * C * K))
    bias = np.ascontiguousarray(b.T)  # [K, J]

    in_maps = []
    for m in range(n_cores):
        xs = x[m * bl : (m + 1) * bl, 1 : J + 1, :]  # [bl, J, D] view
        # -> [J, D, bl] = [J, C*P, bl]
        xt = np.ascontiguousarray(xs.transpose(1, 2, 0))
        if path == "hilo":
            hi, lo = _split_hilo(xt)  # [J, C*P, bl] each
            # planes per j: [hi_c0, hi_c1, lo_c0, lo_c1] along the P-axis
            xt = np.concatenate(
                [hi.reshape(J, C * P, bl), lo.reshape(J, C * P, bl)], axis=1
            )
        in_maps.append({"xt": xt, "wt": wt, "bias": bias})
    return in_maps


def _gather(results, n_cores, bl):
    # per-core out [J, G, K, bl//G]; stripe g, column t = (bh*n_grp + grp)*512 + b
    # holds batch row n*512 + b with n = bh*(G*n_grp) + grp*G + g.
    nb = min(NB_TILE, bl)
    n_bh = bl // nb
    g_n = min(G, nb // N_MM)
    n_grp = (nb // N_MM) // g_n
    out = np.empty((n_cores * bl, J, K), dtype=np.float32)
    for m, r in enumerate(results):
        o = r["out"].reshape(J, g_n, K, n_bh, n_grp, N_MM)
        # -> [bh, grp, g, b512, J, K] -> [bl, J, K]
        o = o.transpose(3, 4, 1, 5, 0, 2).reshape(bl, J, K)
        out[m * bl : (m + 1) * bl] = o
    return out


def _ensure_ntff_hook():
    """The agent image's antenv lacks axon_hooks; shim it so trace=True can
    register the NTFF profiling hook (see trn_agent_boot.trn_boot)."""
    import sys
    import types

    try:
        from antenv.axon_hooks import get_axon_ntff_profile_hook  # noqa: F401

        return
    except ImportError:
        pass
    import antenv

    mod = types.ModuleType("antenv.axon_hooks")
    mod._hook = None

    def set_axon_ntff_profile_hook(h):
        mod._hook = h

    def get_axon_ntff_profile_hook():
        return mod._hook

    mod.set_axon_ntff_profile_hook = set_axon_ntff_profile_hook
    mod.get_axon_ntff_profile_hook = get_axon_ntff_profile_hook
    sys.modules["antenv.axon_hooks"] = mod
    antenv.axon_hooks = mod
    try:
        from trn_agent_boot.trn_boot import _ntff_profile_via_ctypes

        hook = _ntff_profile_via_ctypes("/opt/axon/libaxon_pjrt.so")
        if hook is not None:
            mod._hook = hook
    except Exception:
        pass


def run(x, W, b, path=None, trace=False, n_cores=M_CORES, bl=None):
    from concourse.bass_utils import run_bass_kernel_spmd

    if trace:
        _ensure_ntff_hook()

    path = path or _PATH
    bl = bl or (x.shape[0] // n_cores)
    x = np.asarray(x, dtype=np.float32)
    W = np.asarray(W, dtype=np.float32)
    b = np.asarray(b, dtype=np.float32)
    nc = _get_nc(path, bl)
    in_maps = _prep_core_inputs(x, W, b, path, n_cores, bl)
    res = run_bass_kernel_spmd(
        nc, in_maps, core_ids=list(range(n_cores)), trace=trace
    )
    out = _gather(res.results, n_cores, bl)
    return out, res


def kernel(x, W, b):
    out, _ = run(x, W, b)
    return out


# revision 11
# speedup vs baseline: 1.2441x; 1.0002x over previous
"""Trainium2 Bass kernel for nn_ActionDetokenizer (gnn_message_passing).

Computes: out[b, j, k] = sum_d x[b, j+1, d] * W[j, d, k] + bias[j, k]
  x: [65536, 13, 256] f32, W: [12, 256, 2] f32, b: [12, 2] f32 -> out [65536, 12, 2] f32

Strategy (pure data parallel over batch, 8 cores):
  - Host: shard batch across 8 cores; per core, relayout the needed slice of x
    to d-major [12, C*128, 8192] so the contraction dim (d) lands on SBUF
    partitions (the TensorEngine contracts along partitions).  The tiny weight
    stack is replicated to every core.
  - Device: stream x tiles HBM->SBUF (memory-bound: ~100.7 MB/core) as one
    2 MiB DMA per (joint, batch-chunk); for each (joint, 512-batch column
    chunk) accumulate K=128-contraction matmuls into PSUM with W[j] chunks as
    the stationary operand.  ScalarEngine drains PSUM->SBUF fusing the
    per-(j,k) bias add (Identity activation with per-partition bias AP).
    Output is written k-major [12, 2, 8192] per core and re-oriented on the
    host during the gather step.  Output DMAs ride the second HWDGE ring
    (ACT) so they never stall the input ring (SP).

Compute paths (KERNEL_PATH env or _PATH):
  f32  : exact fp32 matmuls (4 cyc/row on PE)
  f32r : float32r matmuls (1 cyc/row at N>=256; ~1.4e-4 scale-rel err on HW)
  hilo : x and W split into bf16 hi+lo on host; 3-term bf16 matmuls
         (xhi@whi + xhi@wlo + xlo@whi), ~5e-6 scale-rel err, 1 cyc/row.
"""

import os

import numpy as np

M_CORES = 8
B_FULL = 65536
BL = B_FULL // M_CORES  # 8192 batch rows per core
J = 12  # joints
D = 256  # embed dim
K = 2  # outputs per joint
P = 128  # SBUF partitions / d-chunk
C = D // P  # 2 d-chunks
NB_TILE = 4096  # batch columns per SBUF x tile
OT_TILE = 2048  # batch columns per output staging tile
N_MM = 512  # batch columns per matmul (fp32 moving-operand max / PSUM bank)
G = 4  # column-tiling stripes (concurrent matmuls at PE col groups 32*g)

_PATH = os.environ.get("KERNEL_PATH", "f32")

_CACHE = {}


def _build(path, bl):
    import concourse.bacc as bacc
    import concourse.mybir as mybir
    from concourse.tile import TileContext

    f32 = mybir.dt.float32
    xdt = {
        "f32": f32,
        "f32r": mybir.dt.float32r,
        "hilo": mybir.dt.bfloat16,
    }[path]
    # number of (term) planes stacked along the x free dim per joint:
    # f32/f32r: C d-chunks; hilo: 2 sources (hi, lo) x C d-chunks
    n_src = 2 if path == "hilo" else 1
    planes = n_src * C

    # Bacc (not plain Bass): its compile() legalizes multi-wait instructions
    # into event semaphores / ldweights waits, which walrus codegen requires
    # (at most one wait command per compute instruction on TRN2).
    nc = bacc.Bacc("TRN2", target_bir_lowering=False, dynamic_dma_scratch_size=2048)

    # x relayout per core: [J, planes*P, bl]
    x_dram = nc.dram_tensor("xt", [J, planes * P, bl], xdt, kind="ExternalInput")
    # weights: [P, J*n_wsrc*C*K]; hilo has whi,wlo stacked along free dim
    n_wsrc = 2 if path == "hilo" else 1
    w_dram = nc.dram_tensor(
        "wt", [P, J * n_wsrc * C * K], xdt, kind="ExternalInput"
    )
    bias_dram = nc.dram_tensor("bias", [K, J], f32, kind="ExternalInput")

    nb = min(NB_TILE, bl)
    assert bl % nb == 0 and nb % N_MM == 0
    n_bh = bl // nb
    n_n = nb // N_MM
    g_n = min(G, n_n)  # stripes actually used

    out_dram = nc.dram_tensor(
        "out", [J, g_n, K, bl // g_n], f32, kind="ExternalOutput"
    )

    # matmul term sequence per (j, n): (w_src, x_src, c)
    if path == "hilo":
        # hi@whi, lo@whi, hi@wlo  (xlo@wlo dropped: ~2^-18 relative)
        terms = [(0, 0), (0, 1), (1, 0)]
    else:
        terms = [(0, 0)]
    mms = [(ws, xs, c) for (ws, xs) in terms for c in range(C)]

    def w_col(j, ws, c):
        # column offset of W chunk (j, ws, c) in w_dram/w_sb
        return ((j * n_wsrc + ws) * C + c) * K

    def x_plane(xs, c):
        return xs * C + c

    with TileContext(nc) as tc:
        with (
            tc.tile_pool(name="wpool", bufs=1) as wpool,
            tc.tile_pool(name="xpool", bufs=3) as xpool,
            tc.tile_pool(name="opool", bufs=3) as opool,
            tc.tile_pool(name="pspool", bufs=8, space="PSUM") as pspool,
        ):
            w_sb = wpool.tile([P, J * n_wsrc * C * K], xdt, tag="w")
            nc.sync.dma_start(out=w_sb[:, :], in_=w_dram[:, :])
            bias_sb = wpool.tile([K, J], f32, tag="bias")
            nc.sync.dma_start(out=bias_sb[:, :], in_=bias_dram[:, :])

            n_grp = n_n // g_n  # n-chunk groups per (j, bh)
            for j in range(J):
                for bh in range(n_bh):
                    # one DMA brings all planes (d-chunks x hi/lo) for this
                    # (j, bh): [P, planes*nb]
                    xt = xpool.tile([P, planes * nb], xdt, tag="x")
                    src = x_dram[j, :, bh * nb : (bh + 1) * nb]
                    # alternate the two HWDGE rings (SP / ACT) so ring
                    # turnaround gaps overlap with the other ring's transfer
                    dma_eng = nc.sync if (j * n_bh + bh) % 2 == 0 else nc.scalar
                    dma_eng.dma_start(
                        out=xt.rearrange("p (pl b) -> p pl b", pl=planes),
                        in_=src.rearrange("(pl p) b -> p pl b", p=P),
                    )
                    # stripe layout: PSUM/SBUF partition rows 32*g hold the
                    # output of n-chunk n = grp*G + g; the G stripes' matmuls
                    # run CONCURRENTLY in disjoint PE column groups.
                    ot = opool.tile([P, n_grp * N_MM], f32, tag="o")
                    for grp in range(n_grp):
                        ps = pspool.tile([P, N_MM], f32, tag="ps")
                        for i, (ws, xs, c) in enumerate(mms):
                            pl = x_plane(xs, c)
                            wc = w_col(j, ws, c)
                            for g in range(g_n):
                                n = grp * g_n + g
                                col = pl * nb + n * N_MM
                                nc.tensor.matmul(
                                    ps[32 * g : 32 * g + K, :],
                                    lhsT=w_sb[:, wc : wc + K],
                                    rhs=xt[:, col : col + N_MM],
                                    start=(i == 0),
                                    stop=(i == len(mms) - 1),
                                    tile_position=(0, 32 * g),
                                )
                        # PSUM -> SBUF with fused per-(j,k) bias add,
                        # alternating ACT / DVE to split the drain work
                        for g in range(g_n):
                            psl = ps[32 * g : 32 * g + K, :]
                            osl = ot[
                                32 * g : 32 * g + K,
                                grp * N_MM : (grp + 1) * N_MM,
                            ]
                            if g % 2 == 0:
                                nc.scalar.activation(
                                    out=osl,
                                    in_=psl,
                                    func=mybir.ActivationFunctionType.Identity,
                                    bias=bias_sb[:, j : j + 1],
                                    scale=1.0,
                                )
                            else:
                                nc.vector.tensor_scalar_add(
                                    out=osl,
                                    in0=psl,
                                    scalar1=bias_sb[:, j : j + 1],
                                )
                    # output DMAs on the ACT HWDGE ring (keeps SP for x):
                    # one per stripe; host unscrambles the stripe layout.
                    for g in range(g_n):
                        nc.sync.dma_start(
                            out=out_dram[
                                j,
                                g,
                                :,
                                bh * n_grp * N_MM : (bh + 1) * n_grp * N_MM,
                            ],
                            in_=ot[32 * g : 32 * g + K, :],
                        )
    nc.compile()
    return nc


def _get_nc(path, bl):
    key = (path, bl)
    if key not in _CACHE:
        _CACHE[key] = _build(path, bl)
    return _CACHE[key]


def _split_hilo(a):
    import ml_dtypes

    hi = a.astype(ml_dtypes.bfloat16)
    lo = (a - hi.astype(np.float32)).astype(ml_dtypes.bfloat16)
    return hi, lo


def _prep_core_inputs(x, W, b, path, n_cores, bl):
    """Shard batch across cores; relayout x slice to [J, planes*P, bl]."""
    # W chunks: [P, J*n_wsrc*C*K], wt[d, ((j*n_wsrc+ws)*C+c)*K + k]
    wt32 = W.reshape(J, C, P, K).transpose(2, 0, 1, 3)  # [P, J, C, K]
    if path == "hilo":
        hi, lo = _split_hilo(np.ascontiguousarray(wt32))  # [P, J, C, K] each
        wt = np.stack([hi, lo], axis=2)  # [P, J, 2, C, K]
        wt = np.ascontiguousarray(wt.reshape(P, J * 2 * C * K))
    else:
        wt = np.ascontiguousarray(wt32.reshape(P, J * C * K))
    bias = np.ascontiguousarray(b.T)  # [K, J]

    in_maps = []
    for m in range(n_cores):
        xs = x[m * bl : (m + 1) * bl, 1 : J + 1, :]  # [bl, J, D] view
        # -> [J, D, bl] = [J, C*P, bl]
        xt = np.ascontiguousarray(xs.transpose(1, 2, 0))
        if path == "hilo":
            hi, lo = _split_hilo(xt)  # [J, C*P, bl] each
            # planes per j: [hi_c0, hi_c1, lo_c0, lo_c1] along the P-axis
            xt = np.concatenate(
                [hi.reshape(J, C * P, bl), lo.reshape(J, C * P, bl)], axis=1
            )
        in_maps.append({"xt": xt, "wt": wt, "bias": bias})
    return in_maps


def _gather(results, n_cores, bl):
    # per-core out [J, G, K, bl//G]; stripe g, column t = (bh*n_grp + grp)*512 + b
    # holds batch row n*512 + b with n = bh*(G*n_grp) + grp*G + g.
    nb = min(NB_TILE, bl)
    n_bh = bl // nb
    g_n = min(G, nb // N_MM)
    n_grp = (nb // N_MM) // g_n
    out = np.empty((n_cores * bl, J, K), dtype=np.float32)
    for m, r in enumerate(results):
        o = r["out"].reshape(J, g_n, K, n_bh, n_grp, N_MM)
        # -> [bh, grp, g, b512, J, K] -> [bl, J, K]
        o = o.transpose(3, 4, 1, 5, 0, 2).reshape(bl, J, K)
        out[m * bl : (m + 1) * bl] = o
    return out


def _ensure_ntff_hook():
    """The agent image's antenv lacks axon_hooks; shim it so trace=True can
    register the NTFF profiling hook (see trn_agent_boot.trn_boot)."""
    import sys
    import types

    try:
        from antenv.axon_hooks import get_axon_ntff_profile_hook  # noqa: F401

        return
    except ImportError:
        pass
    import antenv

    mod = types.ModuleType("antenv.axon_hooks")
    mod._hook = None

    def set_axon_ntff_profile_hook(h):
        mod._hook = h

    def get_axon_ntff_profile_hook():
        return mod._hook

    mod.set_axon_ntff_profile_hook = set_axon_ntff_profile_hook
    mod.get_axon_ntff_profile_hook = get_axon_ntff_profile_hook
    sys.modules["antenv.axon_hooks"] = mod
    antenv.axon_hooks = mod
    try:
        from trn_agent_boot.trn_boot import _ntff_profile_via_ctypes

        hook = _ntff_profile_via_ctypes("/opt/axon/libaxon_pjrt.so")
        if hook is not None:
            mod._hook = hook
    except Exception:
        pass


def run(x, W, b, path=None, trace=False, n_cores=M_CORES, bl=None):
    from concourse.bass_utils import run_bass_kernel_spmd

    if trace:
        _ensure_ntff_hook()

    path = path or _PATH
    bl = bl or (x.shape[0] // n_cores)
    x = np.asarray(x, dtype=np.float32)
    W = np.asarray(W, dtype=np.float32)
    b = np.asarray(b, dtype=np.float32)
    nc = _get_nc(path, bl)
    in_maps = _prep_core_inputs(x, W, b, path, n_cores, bl)
    res = run_bass_kernel_spmd(
        nc, in_maps, core_ids=list(range(n_cores)), trace=trace
    )
    out = _gather(res.results, n_cores, bl)
    return out, res


def kernel(x, W, b):
    out, _ = run(x, W, b)
    return out
